# revision 2
# baseline (speedup 1.0000x reference)
"""Trainium2 Bass kernel for GCN message passing (nn_GCN_38628935860365).

out = PReLU( segment_sum( adj_vals * (x @ W^T + b)[adj_cols], adj_rows ), alpha )

Strategy (8 NeuronCores, SPMD, full inputs in / full output out):
  - Destination-node sharding: core c owns dest rows [c*12500, (c+1)*12500).
  - Phase A (per core): h_c = x_c @ W^T + b on the tensor engine (fp16
    operands, fp32 PSUM accumulation, bias via a K=1 ones-matmul).
  - Phase B: AllGather h -> h_shared [100352, 256] fp16 in 3 chunks
    overlapped with phase A.  h_shared is laid out AllGather-chunk-major so
    each chunk's output is contiguous AND chunk boundaries align with the
    int16 gather ranges -- phase C range-r gathers start as soon as the
    covering AG chunk lands (Tile tracks the DRAM subrange deps), ~170us
    before the full AllGather completes.
  - Phase C (per core): dest rows degree-sorted into 98 tiles of 128.
    Per (tile, 32768-row source range): one GPSIMD dma_gather (4 SWDGE
    queues round-robin) pulls the edges' source rows of h_shared into SBUF;
    per 128-edge chunk a scatter matrix S^T[e, d] = val[e]*(iota[d]==dest[e])
    is built on the vector engine (dual-op tensor_scalar) or the scalar
    engine (Square+Relu pair) -- 13:7 per-chunk interleave so both engines
    feed each tile's PSUM accumulation chain -- and one PE matmul per chunk
    accumulates into the tile's PSUM bank.  PReLU epilogue on the scalar
    engine, fp16 DMA out (cast to fp32 on host).
  - Host un-permutes the degree-sorted rows and concatenates core shards.

Perf notes (measured): the end-to-end pace is set by the SWDGE gather path
(~90-105 GB/s for per-row 512B descriptors; SDMA engines sit half idle --
per-descriptor latency floor).  DVE ops tax it further via the shared
DVE/GpSimd SBUF port, hence the bounded DVE share.  single_packet=True
hard-faults the runtime; keep False.
"""
import math
import sys
import types

import numpy as np

N_NODES = 100000
N_FEATURES = 512
N_HIDDEN = 256
N_EDGES = 3200000
N_CORES = 8
ALPHA = 0.25

SUP = 3            # tiles per super-gather
AG_CHUNKS = 3      # AllGather split
# Scatter-matrix (S) sourcing: scalar-engine builds vs DRAM streaming.
# The vector engine is OFF LIMITS in phase C: its ops grab the shared
# DVE/GpSimd SBUF port and stall SWDGE gather descriptor generation.
DVE_PER_20 = 13    # of every 20 chunks, this many built on DVE (rest ACT)
SINGLE_PACKET = False

_CACHE = {}
TRACE = False
LAST_EXEC_NS = None


def _install_ntff_shim():
    """Make bass_utils' optional trace path importable (harmless if unused)."""
    if "antenv.axon_hooks" in sys.modules:
        return
    mod = types.ModuleType("antenv.axon_hooks")
    mod._hook = None
    mod.set_axon_ntff_profile_hook = lambda h: setattr(mod, "_hook", h)
    mod.get_axon_ntff_profile_hook = lambda: mod._hook
    sys.modules["antenv.axon_hooks"] = mod
    try:
        from trn_agent_boot.trn_boot import _ntff_profile_via_ctypes
        hook = _ntff_profile_via_ctypes("/opt/axon/libaxon_pjrt.so")
        if hook is not None:
            mod.set_axon_ntff_profile_hook(hook)
    except Exception:
        pass


def _reset_device():
    try:
        import ctypes
        import jax
        jax.devices()
        ctypes.CDLL("/opt/axon/libaxon_pjrt.so").axon_reset()
    except Exception:
        pass


def _plan(adj_rows, adj_cols):
    """Shared (core-independent) structure: per-(tile,range) chunk counts,
    per-tile engine assignment, processing order.  Must be identical across
    cores (SPMD)."""
    shard = N_NODES // N_CORES                      # 12500
    p_nodes = ((shard + 127) // 128) * 128          # 12544
    n_tiles = p_nodes // 128                        # 98
    tot_rows = N_CORES * p_nodes                    # 100352
    n_ranges = (tot_rows + 32767) // 32768          # 4
    ranges = [(r * 32768, min(32768, tot_rows - r * 32768))
              for r in range(n_ranges)]
    n_sup = (n_tiles + SUP - 1) // SUP

    core_of_edge = adj_rows // shard
    # h_shared layout is AllGather-chunk-major: chunk q covers local rows
    # [aq, aq1) of every core's shard and occupies the contiguous region
    # [8*aq, 8*aq1) of h_shared.  (8*4096 = 32768 aligns with idx ranges.)
    ag_rows = [0, 4096, 8192, p_nodes]
    c_src = adj_cols // shard
    i_loc = adj_cols % shard
    q = (i_loc >= ag_rows[1]).astype(np.int64) + (i_loc >= ag_rows[2])
    qrows = np.array([ag_rows[1] - ag_rows[0], ag_rows[2] - ag_rows[1],
                      ag_rows[3] - ag_rows[2]], np.int64)
    qbase = np.array([0, N_CORES * ag_rows[1], N_CORES * ag_rows[2]], np.int64)
    qstart = np.array(ag_rows[:3], np.int64)
    hrow_of_col = qbase[q] + c_src * qrows[q] + (i_loc - qstart[q])
    range_of_edge = hrow_of_col >> 15

    # per-core per-(t,r) counts -> shared ncht = ceil(max/128)
    cnt = np.zeros((N_CORES, n_tiles, n_ranges), np.int64)
    pc = []  # per-core cached edge fields
    for c in range(N_CORES):
        m = core_of_edge == c
        rl = adj_rows[m] - c * shard
        deg = np.bincount(rl, minlength=shard)
        order = np.argsort(-deg, kind="stable")
        rank = np.empty(shard, np.int64)
        rank[order] = np.arange(shard)
        er = rank[rl]
        et = er // 128
        dl = (er % 128).astype(np.float32)
        ridx = range_of_edge[m]
        np.add.at(cnt[c], (et, ridx), 1)
        pc.append(dict(mask=m, order=order, et=et, dl=dl, ridx=ridx,
                       i16=(hrow_of_col[m] & 32767).astype(np.int16)))

    cnt_max = cnt.max(axis=0)
    ncht = (cnt_max + 127) // 128
    for t in range(n_tiles):
        if ncht[t].sum() == 0:
            ncht[t][0] = 1

    chunks_of_tile = ncht.sum(axis=1)
    total_chunks = int(chunks_of_tile.sum())

    # per-CHUNK engine interleave: consecutive chunks of a tile alternate
    # between DVE and ACT so both engines feed the tile's PSUM chain in
    # parallel (a per-tile split serializes big tiles on one engine).
    eng_of_chunk = {}
    n_dve = n_sc = n_dma = 0
    dve_col = {}
    sc_col = {}
    dma_col = {}
    dma_start_of_tile = {}
    ci = 0
    for t in range(n_tiles):
        for r in range(n_ranges):
            for k in range(int(ncht[t][r])):
                if ci % 20 < DVE_PER_20:
                    eng_of_chunk[(t, r, k)] = 'v'
                    dve_col[(t, r, k)] = n_dve
                    n_dve += 1
                else:
                    eng_of_chunk[(t, r, k)] = 'a'
                    sc_col[(t, r, k)] = n_sc
                    n_sc += 1
                ci += 1
        dma_start_of_tile[t] = (0, 0)
    n_dve = max(n_dve, 1)
    n_sc = max(n_sc, 1)
    n_dma = max(n_dma, 1)

    # idx layout: per (t, r) block of ncht[t][r] * 128 slots
    idx_off = {}   # (t, r) -> slot offset
    off = 0
    for t in range(n_tiles):
        for r in range(n_ranges):
            idx_off[(t, r)] = off
            off += int(ncht[t][r]) * 128
    ni_total = off

    last_r = {}
    for t in range(n_tiles):
        lr = 0
        for r in range(n_ranges):
            if ncht[t][r] > 0:
                lr = r
        last_r[t] = lr

    return dict(shard=shard, p_nodes=p_nodes, n_tiles=n_tiles,
                tot_rows=tot_rows, n_ranges=n_ranges, ranges=ranges,
                n_sup=n_sup, ncht=ncht, eng_of_chunk=eng_of_chunk,
                n_dve=n_dve, n_sc=n_sc, n_dma=n_dma,
                dve_col=dve_col, sc_col=sc_col, dma_col=dma_col,
                dma_start_of_tile=dma_start_of_tile,
                idx_off=idx_off, ni_total=ni_total, last_r=last_r,
                total_chunks=total_chunks, pc=pc)


def _preprocess(x, adj_rows, adj_cols, adj_vals, W, b, plan):
    F, HID = N_FEATURES, N_HIDDEN
    shard, p_nodes = plan["shard"], plan["p_nodes"]
    n_tiles, n_ranges = plan["n_tiles"], plan["n_ranges"]
    n_sup = plan["n_sup"]
    ncht = plan["ncht"]
    xf = x[0]

    in_maps = []
    for c in range(N_CORES):
        e = plan["pc"][c]
        m = e["mask"]
        et, dl, ridx, i16 = e["et"], e["dl"], e["ridx"], e["i16"]
        vals = adj_vals[m].astype(np.float32)

        # order edges by (t, r, src) once; then walk buckets
        key = (et * n_ranges + ridx) * 32769 + i16
        perm = np.argsort(key, kind="stable")
        et, ridx, i16, dl, vals = (et[perm], ridx[perm], i16[perm],
                                   dl[perm], vals[perm])
        gid = et * n_ranges + ridx
        bc = np.bincount(gid, minlength=n_tiles * n_ranges)
        gstart = np.concatenate([[0], np.cumsum(bc)])[:-1]
        bc = bc.reshape(n_tiles, n_ranges)
        gstart = gstart.reshape(n_tiles, n_ranges)

        idx_flat = np.zeros(plan["ni_total"], np.int16)
        dve_dest = np.full((128, plan["n_dve"]), 200.0, np.float32)
        dve_val = np.zeros((128, plan["n_dve"]), np.float32)
        sc_nd = np.full((128, plan["n_sc"]), -200.0, np.float32)
        sc_nv = np.zeros((128, plan["n_sc"]), np.float32)
        sc_v = np.zeros((128, plan["n_sc"]), np.float32)
        s_dma = np.zeros((plan["n_dma"], 128, 128), np.float32)

        for t in range(n_tiles):
            for r in range(n_ranges):
                nch = int(ncht[t][r])
                if nch == 0:
                    continue
                off = plan["idx_off"][(t, r)]
                n_real = int(bc[t][r])
                g0 = gstart[t][r]
                idx_flat[off: off + n_real] = i16[g0: g0 + n_real]
                dv = np.full(nch * 128, 200.0, np.float32)
                vv = np.zeros(nch * 128, np.float32)
                dv[:n_real] = dl[g0: g0 + n_real]
                vv[:n_real] = vals[g0: g0 + n_real]
                dv = dv.reshape(nch, 128)
                vv = vv.reshape(nch, 128)
                for k in range(nch):
                    if plan["eng_of_chunk"][(t, r, k)] == 'a':
                        j = plan["sc_col"][(t, r, k)]
                        sc_nd[:, j] = -dv[k]
                        sc_nv[:, j] = -vv[k]
                        sc_v[:, j] = vv[k]
                    else:
                        j = plan["dve_col"][(t, r, k)]
                        dve_dest[:, j] = dv[k]
                        dve_val[:, j] = vv[k]
        idx_w = np.tile(idx_flat.reshape(-1, 16).T, (8, 1))

        xs = np.zeros((p_nodes, F), np.float32)
        xs[:shard] = xf[c * shard: (c + 1) * shard]
        in_maps.append({
            "xT": np.ascontiguousarray(xs.T).astype(np.float16),
            "wT": np.ascontiguousarray(W.T).astype(np.float16),
            "bias": np.asarray(b, np.float32).astype(np.float16).reshape(1, HID),
            "idx": np.ascontiguousarray(idx_w),
            "dve_dest": dve_dest, "dve_val": dve_val,
            "sc_nd": sc_nd, "sc_nv": sc_nv, "sc_v": sc_v,
            "s_dma": np.ascontiguousarray(
                s_dma.transpose(1, 0, 2).reshape(128, plan["n_dma"] * 128)
            ).astype(np.float16),
        })
    return in_maps


def _build_kernel(plan):
    from concourse import bacc, mybir
    import concourse.tile as tile
    from concourse.ap import AP

    F16, F32 = mybir.dt.float16, mybir.dt.float32
    I16 = mybir.dt.int16
    I32 = mybir.dt.int32
    AF = mybir.ActivationFunctionType
    F, HID = N_FEATURES, N_HIDDEN
    p_nodes, n_tiles = plan["p_nodes"], plan["n_tiles"]
    n_ranges, ranges = plan["n_ranges"], plan["ranges"]
    n_sup = plan["n_sup"]
    ncht = plan["ncht"]
    last_r = plan["last_r"]
    K_TILES = F // 128

    # pool sizing: max chunks per (t, r) gather; max dma chunks per tile
    max_nch_tr = int(ncht.max())
    max_dma_t = max(1, max(cnt for _, cnt in plan["dma_start_of_tile"].values()))

    nc = bacc.Bacc(None, target_bir_lowering=False, num_devices=N_CORES,
                   num_swdge_queues=4, dynamic_dma_scratch_size=32768)
    xT_t = nc.dram_tensor("xT", [F, p_nodes], F16, kind="ExternalInput")
    wT_t = nc.dram_tensor("wT", [F, HID], F16, kind="ExternalInput")
    bias_t = nc.dram_tensor("bias", [1, HID], F16, kind="ExternalInput")
    idx_t = nc.dram_tensor("idx", [128, plan["ni_total"] // 16], I16,
                           kind="ExternalInput")
    dved_t = nc.dram_tensor("dve_dest", [128, plan["n_dve"]], F32,
                            kind="ExternalInput")
    dvev_t = nc.dram_tensor("dve_val", [128, plan["n_dve"]], F32,
                            kind="ExternalInput")
    scnd_t = nc.dram_tensor("sc_nd", [128, plan["n_sc"]], F32,
                            kind="ExternalInput")
    scnv_t = nc.dram_tensor("sc_nv", [128, plan["n_sc"]], F32,
                            kind="ExternalInput")
    scv_t = nc.dram_tensor("sc_v", [128, plan["n_sc"]], F32,
                           kind="ExternalInput")
    sdma_t = nc.dram_tensor("s_dma", [128, plan["n_dma"] * 128], F16,
                            kind="ExternalInput")
    out_t = nc.dram_tensor("out", [p_nodes, HID], F16, kind="ExternalOutput")

    h_own = nc.dram_tensor("h_own", [p_nodes, HID], F16)
    h_shared = nc.dram_tensor("h_shared", [N_CORES * p_nodes, HID], F16,
                              addr_space="Shared")

    # AllGather chunk row boundaries (slab-aligned)
    ag_rows = [0, 4096, 8192, p_nodes]

    with tile.TileContext(nc) as tc:
        with (
            tc.tile_pool(name="const", bufs=1) as cpool,
            tc.tile_pool(name="xsl", bufs=3) as xpool,
            tc.tile_pool(name="hsb", bufs=4) as hpool,
            tc.tile_pool(name="msgs", bufs=6) as mpool,
            tc.tile_pool(name="st", bufs=12) as spool,
            tc.tile_pool(name="sq", bufs=6) as qpool,
            tc.tile_pool(name="sd", bufs=3) as dpool,
            tc.tile_pool(name="outp", bufs=3) as opool,
            tc.tile_pool(name="ps", bufs=8, space="PSUM") as pps,
        ):
            # ---- constants needed by phase A first ----
            wt_sb = cpool.tile([128, K_TILES, HID], F16)
            for kt in range(K_TILES):
                nc.sync.dma_start(out=wt_sb[:, kt, :],
                                  in_=wT_t[kt * 128: (kt + 1) * 128, :])
            ones_sb = cpool.tile([1, 128], F16)
            nc.vector.memset(ones_sb[:], 1.0)
            bias_sb = cpool.tile([1, HID], F16)
            nc.sync.dma_start(out=bias_sb[:], in_=bias_t[:, :])

            iota_i32 = cpool.tile([128, 128], I32)
            nc.gpsimd.iota(iota_i32[:], pattern=[[1, 128]], base=0,
                           channel_multiplier=0)
            iota_f16 = cpool.tile([128, 128], F16)
            nc.vector.tensor_copy(iota_f16[:], iota_i32[:])

            # ---- phase A (+ chunked AllGather) ----
            SLAB = 512
            ag_done = 0
            for sl in range(math.ceil(p_nodes / SLAB)):
                w = min(SLAB, p_nodes - sl * SLAB)
                xsl = xpool.tile([128, K_TILES, SLAB], F16, tag="xsl")
                for kt in range(K_TILES):
                    nc.sync.dma_start(
                        out=xsl[:, kt, :w],
                        in_=xT_t[kt * 128: (kt + 1) * 128,
                                 sl * SLAB: sl * SLAB + w])
                for j in range(w // 128):
                    psum_h = pps.tile([128, HID], F32, space="PSUM", tag="ps")
                    for kt in range(K_TILES):
                        nc.tensor.matmul(
                            psum_h[:], lhsT=xsl[:, kt, j * 128: (j + 1) * 128],
                            rhs=wt_sb[:, kt, :], start=(kt == 0), stop=False)
                    nc.tensor.matmul(psum_h[:], lhsT=ones_sb[:], rhs=bias_sb[:],
                                     start=False, stop=True)
                    h_sb = hpool.tile([128, HID], F16, tag="hsb")
                    nc.scalar.activation(h_sb[:], psum_h[:], func=AF.Copy)
                    r0 = sl * SLAB + j * 128
                    nc.sync.dma_start(out=h_own[r0: r0 + 128, :], in_=h_sb[:])
                # fire AllGather chunks as soon as their rows are done
                done_rows = sl * SLAB + w
                while ag_done < AG_CHUNKS and done_rows >= ag_rows[ag_done + 1]:
                    a0, a1 = ag_rows[ag_done], ag_rows[ag_done + 1]
                    g0 = N_CORES * a0
                    nc.gpsimd.collective_compute(
                        "AllGather", mybir.AluOpType.bypass,
                        replica_groups=[list(range(N_CORES))],
                        ins=[h_own[a0:a1, :].opt()],
                        outs=[h_shared[g0: g0 + N_CORES * (a1 - a0), :].opt()],
                    )
                    ag_done += 1

            # ---- phase C tables (loaded while phase A runs) ----
            idx_sb = cpool.tile([128, plan["ni_total"] // 16], I16)
            nc.sync.dma_start(out=idx_sb[:], in_=idx_t[:, :])
            dved_sb = cpool.tile([128, plan["n_dve"]], F32)
            nc.sync.dma_start(out=dved_sb[:], in_=dved_t[:, :])
            dvev_sb = cpool.tile([128, plan["n_dve"]], F32)
            nc.sync.dma_start(out=dvev_sb[:], in_=dvev_t[:, :])
            scnd_sb = cpool.tile([128, plan["n_sc"]], F32)
            nc.sync.dma_start(out=scnd_sb[:], in_=scnd_t[:, :])
            scnv_sb = cpool.tile([128, plan["n_sc"]], F32)
            nc.sync.dma_start(out=scnv_sb[:], in_=scnv_t[:, :])
            scv_sb = cpool.tile([128, plan["n_sc"]], F32)
            nc.sync.dma_start(out=scv_sb[:], in_=scv_t[:, :])

            # ---- phase C ----
            g_ctr = 0
            for t in range(n_tiles):
                d0, nd_t = plan["dma_start_of_tile"][t]
                sd = None
                if nd_t > 0:
                    sd = dpool.tile([128, max_dma_t, 128], F16, tag="sd")
                    nc.sync.dma_start(
                        out=sd[:, :nd_t, :],
                        in_=sdma_t[:, d0 * 128: (d0 + nd_t) * 128])
                psum_t = pps.tile([128, HID], F32, space="PSUM", tag="ps")
                started = False
                for r in range(n_ranges):
                    nch = int(ncht[t][r])
                    if nch == 0:
                        continue
                    rbase, rlen = ranges[r]
                    off = plan["idx_off"][(t, r)]
                    msgs = mpool.tile([128, max_nch_tr, HID], F16, tag="msgs")
                    nc.gpsimd.dma_gather(
                        out_ap=msgs[:, :nch, :],
                        in_ap=h_shared[rbase: rbase + rlen, :],
                        idxs_ap=idx_sb[:, off // 16: (off + nch * 128) // 16],
                        num_idxs=nch * 128,
                        num_idxs_reg=nch * 128,
                        elem_size=HID,
                        single_packet=SINGLE_PACKET,
                        queue_num=g_ctr % 4,
                    )
                    g_ctr += 1
                    for k in range(nch):
                        if plan["eng_of_chunk"][(t, r, k)] == 'v':
                            j = plan["dve_col"][(t, r, k)]
                            s_t = spool.tile([128, 128], F16, tag="st")
                            nc.vector.tensor_scalar(
                                s_t[:], iota_f16[:],
                                dved_sb[:, j: j + 1], dvev_sb[:, j: j + 1],
                                op0=mybir.AluOpType.is_equal,
                                op1=mybir.AluOpType.mult)
                            s_ap = s_t[:]
                        elif plan["eng_of_chunk"][(t, r, k)] == 'a':
                            j = plan["sc_col"][(t, r, k)]
                            s_t = spool.tile([128, 128], F16, tag="st")
                            sq_t = qpool.tile([128, 128], F16, tag="sq")
                            nc.scalar.activation(
                                sq_t[:], iota_f16[:], func=AF.Square,
                                bias=scnd_sb[:, j: j + 1])
                            nc.scalar.activation(
                                s_t[:], sq_t[:], func=AF.Relu,
                                scale=scnv_sb[:, j: j + 1],
                                bias=scv_sb[:, j: j + 1])
                            s_ap = s_t[:]
                        stop = (r == last_r[t]) and (k == nch - 1)
                        nc.tensor.matmul(
                            psum_t[:], lhsT=s_ap,
                            rhs=msgs[:, k, :],
                            start=not started, stop=stop)
                        started = True
                out_sb = opool.tile([128, HID], F16, tag="out")
                nc.scalar.activation(out_sb[:], psum_t[:],
                                     func=AF.Prelu, alpha=ALPHA)
                nc.sync.dma_start(out=out_t[t * 128: (t + 1) * 128, :],
                                  in_=out_sb[:])
    nc.finalize()
    return nc


def kernel(x, adj_rows, adj_cols, adj_vals, W, b, alpha):
    x = np.asarray(x, np.float32)
    adj_rows = np.asarray(adj_rows, np.int64)
    adj_cols = np.asarray(adj_cols, np.int64)
    adj_vals = np.asarray(adj_vals, np.float32)
    W = np.asarray(W, np.float32)
    b = np.asarray(b, np.float32)

    _install_ntff_shim()
    _reset_device()
    from concourse.bass_utils import run_bass_kernel_spmd

    plan = _plan(adj_rows, adj_cols)
    in_maps = _preprocess(x, adj_rows, adj_cols, adj_vals, W, b, plan)
    key = ("gcn8v7", SINGLE_PACKET, DVE_PER_20,
           plan["total_chunks"], plan["ni_total"],
           tuple(plan["ncht"].ravel()))
    if key not in _CACHE:
        _CACHE[key] = _build_kernel(plan)
    nc = _CACHE[key]
    global LAST_EXEC_NS
    res = run_bass_kernel_spmd(nc, in_maps, core_ids=list(range(N_CORES)),
                               trace=TRACE)
    LAST_EXEC_NS = res.exec_time_ns

    out = np.empty((1, N_NODES, N_HIDDEN), np.float32)
    shard = plan["shard"]
    for c in range(N_CORES):
        oc = res.results[c]["out"]
        order = plan["pc"][c]["order"]
        out[0, c * shard + order] = oc[:shard]
    return out
